# revision 44
# baseline (speedup 1.0000x reference)
"""Trainium2 Bass kernel for CRF negative log-likelihood (nn_CRF).

Math (reference semantics, tags always valid in [0,128)):
  nll = -mean_b(scores[b] - log_z[b]) / 100

  scores[b] = sum_s em[b,s,tag_s]                       (device, PE diag-gather)
            + T[BOS,tag_0] + sum_s T[tag_{s-1},tag_s] + T[tag_last,EOS]
                                                        (host fp64: tags+T only)
  log_z[b]  = forward algorithm over the 128 real labels (BOS/EOS rows/cols
              are exactly unreachable: exp(-10000) == 0 in fp32).

Device strategy (8 cores x 6 chains = 48 sequence chunks of 42-43 steps):
  * Forward recursion in the exp domain: q <- (q @ expT) * exp(em_s - K) with
    constant per-step rescale exp(-K). Each chunk starts from a uniform vector
    with 5-6 warmup steps (the dense random CRF forward map contracts ~0.1x
    per step), so chunk log-gains telescope:
      log_z = phi_end(chunk0) + sum_{others}(phi_end - phi_pre) + 2047*K.
    Chunk 0 of core 0 gets the exact initial state u0 = exp(em_0 + T[BOS,:])
    blended in via a data-driven gamma scalar.
  * The 6 chains form 3 groups of 2; each group's two chains share one
    [128,512] matmul + one fused DVE tensor_tensor (q = ps * exc, PSUM read)
    per step; the three groups ping-pong to hide the PE -> DVE -> PE chain
    latency.
  * exp(em - K) runs on ScalarE, prefetched one chunk ahead (half-chunk ops
    mid-stream; quarter-chunk ops at startup so all three groups start asap,
    with a dummy activation first to hide the ACT table load). Emissions
    stream in as fp8; sa/sb DMAs prefetch one chunk ahead so neither the exp
    nor the in-order PE queue waits on HBM.
  * Gold-path emission score via PE DoubleRow diag-accumulate: one-hot masks
    M_s (fp8) paired over two steps as stationary weights against the same
    emission stream the scan reads:
      dacc_h[b',b] += sum_l sum_j M_(s+j)[l,b'] em_(s+j)[l,b]
    whose diagonal is the per-batch emission score sum.

The program is fully SPMD: per-core differences ride in the input data
(zero-padded warmup slices, gamma blend scalars, BOS bias columns, final
functional vectors).
"""
import sys, os

for _p in ("/opt/trn_rl_repo",):
    if _p not in sys.path and os.path.isdir(_p):
        sys.path.insert(0, _p)

import numpy as np
import ml_dtypes

B, S, NL = 256, 2048, 128
NB, BOS, EOS = 130, 128, 129
NCORES = 8
NCHAIN = 6             # chains per core (3 groups of 2)
NGRP = 3
TILES = 48             # slots per chain
CHUNK = 8              # slots per DMA/exp chunk
NCH = TILES // CHUNK   # 6 chunks per chain
LCH = [43, 43, 43, 43, 42, 42]      # real steps per chain (sum 256)
OFF = [0, 43, 86, 129, 172, 214]    # chain start offsets within a core
WG = [5, 5, 6]                      # warmup slots per group (TILES - L)
F8 = ml_dtypes.float8_e4m3
BF16 = ml_dtypes.bfloat16

LOG2E = 1.4426950408889634
SIGMA = -0.0574        # mantissa-linear exp2 bias centering

_prog_cache = {}


def _estimate_K(em, T):
    """Mean per-step log-growth of the forward recursion (host, tiny presim)."""
    expT = np.exp(T[:NL, :NL].astype(np.float64))
    nb = 4
    v = np.exp(T[BOS, :NL].astype(np.float64)[None, :] + em[:nb, 0, :].astype(np.float64))
    g = []
    for s in range(1, 33):
        v = (v @ expT) * np.exp(em[:nb, s, :].astype(np.float64))
        n = v.sum(axis=1)
        g.append(np.log(n))
        v /= n[:, None]
    g = np.array(g[8:])  # skip mixing transient
    return float(g.mean())


def _host_prep(emissions, tags, transitions):
    em = np.asarray(emissions, np.float32)
    tg = np.asarray(tags, np.int64)
    T = np.asarray(transitions, np.float32)

    K = _estimate_K(em, T)
    expT_bf = np.exp(T[:NL, :NL]).astype(BF16)            # [prev, cur]
    teos_bf = np.exp(T[:NL, EOS]).astype(BF16)

    em_t = np.ascontiguousarray(em.transpose(1, 2, 0)).astype(F8)     # [S, 128, B]
    M = np.zeros((S, NL, B), F8)
    M[np.arange(S)[:, None], tg.T, np.arange(B)[None, :]] = 1.0

    # transition part of the gold score: host fp64, touches only tags + T
    T64 = T.astype(np.float64)
    trans_sc = (T64[BOS, tg[:, 0]]
                + T64[tg[:, :-1], tg[:, 1:]].sum(axis=1)
                + T64[tg[:, -1], EOS])                                 # [B]

    in_maps = []
    for k in range(NCORES):
        sa = np.zeros((NCHAIN, TILES, NL, B), F8)
        sb = np.zeros((NCHAIN, TILES, NL, B), F8)
        gam = np.ones((NL, NCHAIN), np.float32)
        tbos = np.full((NL, NCHAIN), -10000.0, np.float32)
        for ch in range(NCHAIN):
            s0 = 256 * k + OFF[ch]
            w = WG[ch // 2]
            for j in range(TILES):
                s = s0 - w + j
                if s >= 0:
                    sa[ch, j] = em_t[s]
                if j >= w:
                    sb[ch, j] = M[s]
            if k == 0 and ch == 0:
                gam[:, 0] = 0.0
                tbos[:, 0] = T[BOS, :NL]
        # column order per slot-chunk: [g, t8, chp, b]; slot-major 3D view
        sa = sa.reshape(NGRP, 2, NCH, CHUNK, NL, B)
        sb = sb.reshape(NGRP, 2, NCH, CHUNK, NL, B)
        sa = np.ascontiguousarray(sa.transpose(2, 4, 0, 3, 1, 5)).reshape(NCH, NL, NGRP * CHUNK * 2 * B)
        sb = np.ascontiguousarray(sb.transpose(2, 4, 0, 3, 1, 5)).reshape(NCH, NL, NGRP * CHUNK * 2 * B)

        cb = np.zeros((NL, 2 * NL + 1 + NCHAIN), BF16)
        cb[:, 0:NL] = expT_bf
        cb[:, NL:2 * NL] = np.eye(NL, dtype=BF16)
        cb[:, 2 * NL:2 * NL + 1] = 1.0
        for ch in range(NCHAIN):
            last = (k == NCORES - 1 and ch == NCHAIN - 1)
            cb[:, 2 * NL + 1 + ch] = teos_bf if last else np.ones(NL, BF16)
        cf = np.zeros((NL, 2 * NCHAIN), np.float32)
        cf[:, 0:NCHAIN] = gam
        cf[:, NCHAIN:2 * NCHAIN] = tbos
        in_maps.append({"sa": sa, "sb": sb, "cbf": cb, "cfp": cf})
    return in_maps, K, trans_sc


def _build_program(K):
    import contextlib
    import concourse.bass as bass
    import concourse.tile as tile
    from concourse import bacc, mybir

    dt = mybir.dt
    Alu = mybir.AluOpType
    Act = mybir.ActivationFunctionType
    DR = mybir.MatmulPerfMode.DoubleRow

    GW = NGRP * CHUNK          # slots per chunk across groups
    SLOTW = 2 * B              # columns per slot

    nc = bacc.Bacc("TRN2", target_bir_lowering=False, debug=False, num_devices=NCORES)

    sa_d = nc.dram_tensor("sa", [NCH, NL, GW * SLOTW], dt.float8e4, kind="ExternalInput").ap()
    sb_d = nc.dram_tensor("sb", [NCH, NL, GW * SLOTW], dt.float8e4, kind="ExternalInput").ap()
    cbf_d = nc.dram_tensor("cbf", [NL, 2 * NL + 1 + NCHAIN], dt.bfloat16, kind="ExternalInput").ap()
    cfp_d = nc.dram_tensor("cfp", [NL, 2 * NCHAIN], dt.float32, kind="ExternalInput").ap()

    phis_d = nc.dram_tensor("phis", [1, NCHAIN * 2 * B], dt.float32, kind="ExternalOutput").ap()
    etpart_d = nc.dram_tensor("etpart", [NL, 2], dt.float32, kind="ExternalOutput").ap()

    with tile.TileContext(nc) as tc:
        with contextlib.ExitStack() as ctx:
            const = ctx.enter_context(tc.tile_pool(name="const", bufs=1))
            emring = ctx.enter_context(tc.tile_pool(name="emring", bufs=3))
            exring = ctx.enter_context(tc.tile_pool(name="exring", bufs=3))
            dring = ctx.enter_context(tc.tile_pool(name="dring", bufs=3))
            ps = ctx.enter_context(tc.tile_pool(name="ps", bufs=1, space="PSUM"))
            php = ctx.enter_context(tc.tile_pool(name="php", bufs=2, space="PSUM"))

            negK = const.tile([NL, 1], dt.float32)
            nc.vector.memset(negK[:], -K)
            actwarm = const.tile([NL, 1], dt.float32)
            nc.scalar.activation(actwarm[:], negK[:], Act.Exp, bias=0.0, scale=1.0)

            cbf = const.tile([NL, 2 * NL + 1 + NCHAIN], dt.bfloat16)
            cfp = const.tile([NL, 2 * NCHAIN], dt.float32)
            expT = cbf[:, 0:NL]
            ident = cbf[:, NL:2 * NL]
            fones = cbf[:, 2 * NL:2 * NL + 1]
            fvec = cbf[:, 2 * NL + 1:2 * NL + 1 + NCHAIN]
            gam = cfp[:, 0:NCHAIN]
            tbos = cfp[:, NCHAIN:2 * NCHAIN]

            q01 = const.tile([NL, 2 * SLOTW], dt.bfloat16, name="q01")
            nc.vector.memset(q01[:], 1.0)
            q2 = const.tile([NL, SLOTW], dt.bfloat16, name="q2")
            nc.vector.memset(q2[:], 1.0)
            qg = [q01[:, 0:SLOTW], q01[:, SLOTW:2 * SLOTW], q2[:]]
            us = [const.tile([NL, B], dt.bfloat16, name=f"u{ch}") for ch in range(NCHAIN)]

            ps01 = ps.tile([NL, 2 * SLOTW], dt.float32, name="ps01")
            ps2 = ps.tile([NL, SLOTW], dt.float32, name="ps2")
            psg = [ps01[:, 0:SLOTW], ps01[:, SLOTW:2 * SLOTW], ps2[:]]
            dacc = [ps.tile([NL, NL], dt.float32, name=f"dacc{h}") for h in range(2)]
            phi_sb = const.tile([1, NCHAIN * 2 * B], dt.float32)

            n_dacc = NCH * (CHUNK // 2) * NCHAIN * 2  # DR matmuls total
            i_dacc = 0

            sa_ts, exc_ts = {}, {}

            def fetch_sa(c):
                sa_t = emring.tile([NL, GW * SLOTW], dt.float8e4, name=f"sa{c}", tag="em")
                for g in range(NGRP):
                    sl = slice(g * CHUNK * SLOTW, (g + 1) * CHUNK * SLOTW)
                    nc.sync.dma_start(sa_t[:, sl], sa_d[c, :, sl])
                sa_ts[c] = sa_t
                exc_ts[c] = exring.tile([NL, GW * SLOTW], dt.bfloat16, name=f"ex{c}", tag="ex")

            HSL = CHUNK * SLOTW // 2   # half of a group's chunk block

            def exp_half(c, g, h):
                sl = slice(g * CHUNK * SLOTW + h * HSL, g * CHUNK * SLOTW + (h + 1) * HSL)
                nc.scalar.activation(exc_ts[c][:, sl], sa_ts[c][:, sl],
                                     Act.Exp, bias=negK[:], scale=1.0)

            sb_ts = {}

            def fetch_sb(c):
                sb_t = dring.tile([NL, GW, SLOTW], dt.float8e4, name=f"sb{c}", tag="d")
                nc.sync.dma_start(sb_t[:, :, :], sb_d[c].rearrange("p (w q) -> p w q", w=GW))
                sb_ts[c] = sb_t

            QSL = CHUNK * SLOTW // 4   # quarter of a group's chunk block

            def exp_quarter(c, g, qi):
                sl = slice(g * CHUNK * SLOTW + qi * QSL, g * CHUNK * SLOTW + (qi + 1) * QSL)
                nc.scalar.activation(exc_ts[c][:, sl], sa_ts[c][:, sl],
                                     Act.Exp, bias=negK[:], scale=1.0)

            fetch_sa(0)
            nc.sync.dma_start(cbf[:], cbf_d[:])
            nc.sync.dma_start(cfp[:], cfp_d[:])
            fetch_sb(0)
            # first quarter of every group first, so all three chains start asap;
            # then the remaining three quarters of each group as one op.
            for g in range(NGRP):
                exp_quarter(0, g, 0)
            for ch in range(NCHAIN):
                g, chp = ch // 2, ch % 2
                off = (g * CHUNK + WG[g]) * SLOTW + chp * B
                nc.scalar.activation(us[ch][:], sa_ts[0][:, off:off + B],
                                     Act.Exp, bias=tbos[:, ch:ch + 1], scale=1.0)
            for g in range(NGRP):
                sl = slice(g * CHUNK * SLOTW + QSL, (g + 1) * CHUNK * SLOTW)
                nc.scalar.activation(exc_ts[0][:, sl], sa_ts[0][:, sl],
                                     Act.Exp, bias=negK[:], scale=1.0)

            for c in range(NCH):
                sa_t, exc = sa_ts[c], exc_ts[c]
                sa3 = sa_t[:].rearrange("p (w q) -> p w q", q=SLOTW)
                sb_t = sb_ts[c]
                if c + 1 < NCH:
                    fetch_sa(c + 1)
                    fetch_sb(c + 1)

                exc3 = exc[:].rearrange("p (g x) -> p g x", g=NGRP)
                q013 = q01[:].rearrange("p (g x) -> p g x", g=2)
                ps013 = ps01[:].rearrange("p (g x) -> p g x", g=2)
                for t8 in range(CHUNK):
                    t = c * CHUNK + t8
                    for g in range(NGRP):
                        q = qg[g]
                        if t == WG[g]:
                            for chp in range(2):
                                ch = 2 * g + chp
                                pht = php.tile([1, B], dt.float32, name=f"php{ch}", tag="ph")
                                nc.tensor.matmul(pht[:], fones[:],
                                                 q[:, chp * B:(chp + 1) * B],
                                                 start=True, stop=True)
                                nc.scalar.copy(phi_sb[:, ch * B:(ch + 1) * B], pht[:])
                        nc.tensor.matmul(psg[g], expT[:], q[:], start=True, stop=True)
                        if g == 1:
                            # paired multiply for groups 0+1: one [128,2,512] TT
                            nc.vector.tensor_tensor(
                                q013, ps013,
                                exc3[:, 0:2, t8 * SLOTW:(t8 + 1) * SLOTW], Alu.mult)
                        elif g == 2:
                            exs = exc[:, (g * CHUNK + t8) * SLOTW:(g * CHUNK + t8 + 1) * SLOTW]
                            nc.vector.tensor_tensor(q[:], psg[g], exs, Alu.mult)
                    for g in range(NGRP):
                        if c * CHUNK + t8 == WG[g]:
                            for chp in range(2):
                                ch = 2 * g + chp
                                qh = qg[g][:, chp * B:(chp + 1) * B]
                                nc.vector.scalar_tensor_tensor(
                                    qh, qh, gam[:, ch:ch + 1],
                                    us[ch][:], Alu.mult, Alu.add)
                    if c + 1 < NCH and t8 in (1, 2, 3, 4, 5, 6):
                        exp_half(c + 1, (t8 - 1) // 2, (t8 - 1) % 2)
                    if t8 % 2 == 1:
                        # DoubleRow diag accumulate over the (t8-1, t8) slot pair
                        sb3 = sb_t
                        for g in range(NGRP):
                            for chp in range(2):
                                for h in range(2):
                                    csl = slice(chp * B + h * NL, chp * B + (h + 1) * NL)
                                    nc.tensor.matmul(
                                        dacc[h][:],
                                        sb3[:, g * CHUNK + t8 - 1: g * CHUNK + t8 + 1, csl],
                                        sa3[:, g * CHUNK + t8 - 1: g * CHUNK + t8 + 1, csl],
                                        start=(i_dacc == 0), stop=(i_dacc == n_dacc - 1),
                                        perf_mode=DR)
                                i_dacc += 1

            # gold-score diag extract first (dacc completes at the last t8)
            escr = const.tile([NL, NL], dt.bfloat16)
            etp = const.tile([NL, 2], dt.float32)
            for h in range(2):
                nc.vector.scalar_tensor_tensor(escr[:], dacc[h][:], 1.0, ident[:],
                                               Alu.mult, Alu.mult, accum_out=etp[:, h:h + 1])
            nc.sync.dma_start(etpart_d[:], etp[:])

            for ch in range(NCHAIN):
                g, chp = ch // 2, ch % 2
                pht = php.tile([1, B], dt.float32, name=f"phe{ch}", tag="ph")
                nc.tensor.matmul(pht[:], fvec[:, ch:ch + 1],
                                 qg[g][:, chp * B:(chp + 1) * B], start=True, stop=True)
                nc.vector.tensor_copy(phi_sb[:, (NCHAIN + ch) * B:(NCHAIN + ch + 1) * B],
                                      pht[:])
            nc.sync.dma_start(phis_d[:], phi_sb[:])

    nc.compile()
    return nc


def run(emissions, tags, transitions, trace=False, trace_cores=None):
    from concourse.bass_utils import run_bass_kernel_spmd
    in_maps, K, trans_sc = _host_prep(emissions, tags, transitions)
    key = f"{K:.9f}"
    if key not in _prog_cache:
        _prog_cache[key] = _build_program(K)
    nc = _prog_cache[key]
    r = run_bass_kernel_spmd(nc, in_maps, list(range(NCORES)), trace=trace,
                             trace_cores=trace_cores)

    # phis per core: [pre0..pre5 | end0..end5] each [B]
    raw = np.stack([r.results[k]["phis"].reshape(2 * NCHAIN, B) for k in range(NCORES)])
    raw = np.log(raw.astype(np.float64))
    pre = raw[:, 0:NCHAIN].reshape(NCORES * NCHAIN, B)
    end = raw[:, NCHAIN:2 * NCHAIN].reshape(NCORES * NCHAIN, B)
    log_z = end[0] + end[1:].sum(0) - pre[1:].sum(0) + 2047.0 * K

    etp = np.stack([r.results[k]["etpart"] for k in range(NCORES)]).sum(0)  # [128, 2]
    em_sc = etp.transpose(1, 0).reshape(2 * NL).astype(np.float64)          # [B]
    scores = em_sc + trans_sc
    nll = -np.mean(scores - log_z) / 100.0
    return np.float32(nll), r


def kernel(emissions, tags, transitions):
    out, _ = run(emissions, tags, transitions, trace=False)
    return out


# revision 45
# speedup vs baseline: 1.1420x; 1.1420x over previous
"""Trainium2 Bass kernel for CRF negative log-likelihood (nn_CRF).

Math (reference semantics, tags always valid in [0,128)):
  nll = -mean_b(scores[b] - log_z[b]) / 100

  scores[b] = sum_s em[b,s,tag_s]                       (device, PE diag-gather)
            + T[BOS,tag_0] + sum_s T[tag_{s-1},tag_s] + T[tag_last,EOS]
                                                        (host fp64: tags+T only)
  log_z[b]  = forward algorithm over the 128 real labels (BOS/EOS rows/cols
              are exactly unreachable: exp(-10000) == 0 in fp32).

Device strategy (8 cores x 6 chains = 48 sequence chunks of 42-43 steps):
  * Forward recursion in the exp domain: q <- (q @ expT) * exp(em_s - K) with
    constant per-step rescale exp(-K). Each chunk starts from a uniform vector
    with 5-6 warmup steps (the dense random CRF forward map contracts ~0.1x
    per step), so chunk log-gains telescope:
      log_z = phi_end(chunk0) + sum_{others}(phi_end - phi_pre) + 2047*K.
    Chunk 0 of core 0 gets the exact initial state u0 = exp(em_0 + T[BOS,:])
    blended in via a data-driven gamma scalar.
  * The 6 chains form 3 groups of 2; each group's two chains share one
    [128,512] matmul + one fused DVE tensor_tensor (q = ps * exc, PSUM read)
    per step; the three groups ping-pong to hide the PE -> DVE -> PE chain
    latency.
  * exp(em - K) runs on ScalarE, prefetched one chunk ahead (half-chunk ops
    mid-stream; quarter-chunk ops at startup so all three groups start asap,
    with a dummy activation first to hide the ACT table load). Emissions
    stream in as fp8; sa/sb DMAs prefetch one chunk ahead so neither the exp
    nor the in-order PE queue waits on HBM.
  * Gold-path emission score via PE DoubleRow diag-accumulate: one-hot masks
    M_s (fp8) paired over two steps as stationary weights against the same
    emission stream the scan reads:
      dacc_h[b',b] += sum_l sum_j M_(s+j)[l,b'] em_(s+j)[l,b]
    whose diagonal is the per-batch emission score sum.

The program is fully SPMD: per-core differences ride in the input data
(zero-padded warmup slices, gamma blend scalars, BOS bias columns, final
functional vectors).
"""
import sys, os

for _p in ("/opt/trn_rl_repo",):
    if _p not in sys.path and os.path.isdir(_p):
        sys.path.insert(0, _p)

import numpy as np
import ml_dtypes

B, S, NL = 256, 2048, 128
NB, BOS, EOS = 130, 128, 129
NCORES = 8
NCHAIN = 6             # chains per core (3 groups of 2)
NGRP = 3
TILES = 48             # slots per chain
CHUNK = 8              # slots per DMA/exp chunk
NCH = TILES // CHUNK   # 6 chunks per chain
LCH = [43, 43, 43, 43, 42, 42]      # real steps per chain (sum 256)
OFF = [0, 43, 86, 129, 172, 214]    # chain start offsets within a core
WG = [5, 5, 6]                      # warmup slots per group (TILES - L)
F8 = ml_dtypes.float8_e4m3
BF16 = ml_dtypes.bfloat16

LOG2E = 1.4426950408889634
SIGMA = -0.0574        # mantissa-linear exp2 bias centering

_prog_cache = {}


def _estimate_K(em, T):
    """Mean per-step log-growth of the forward recursion (host, tiny presim)."""
    expT = np.exp(T[:NL, :NL].astype(np.float64))
    nb = 4
    v = np.exp(T[BOS, :NL].astype(np.float64)[None, :] + em[:nb, 0, :].astype(np.float64))
    g = []
    for s in range(1, 33):
        v = (v @ expT) * np.exp(em[:nb, s, :].astype(np.float64))
        n = v.sum(axis=1)
        g.append(np.log(n))
        v /= n[:, None]
    g = np.array(g[8:])  # skip mixing transient
    return float(g.mean())


def _host_prep(emissions, tags, transitions):
    em = np.asarray(emissions, np.float32)
    tg = np.asarray(tags, np.int64)
    T = np.asarray(transitions, np.float32)

    K = _estimate_K(em, T)
    expT_bf = np.exp(T[:NL, :NL]).astype(BF16)            # [prev, cur]
    teos_bf = np.exp(T[:NL, EOS]).astype(BF16)

    em_t = np.ascontiguousarray(em.transpose(1, 2, 0)).astype(F8)     # [S, 128, B]
    M = np.zeros((S, NL, B), F8)
    M[np.arange(S)[:, None], tg.T, np.arange(B)[None, :]] = 1.0

    # transition part of the gold score: host fp64, touches only tags + T
    T64 = T.astype(np.float64)
    trans_sc = (T64[BOS, tg[:, 0]]
                + T64[tg[:, :-1], tg[:, 1:]].sum(axis=1)
                + T64[tg[:, -1], EOS])                                 # [B]

    in_maps = []
    for k in range(NCORES):
        sa = np.zeros((NCHAIN, TILES, NL, B), F8)
        sb = np.zeros((NCHAIN, TILES, NL, B), F8)
        gam = np.ones((NL, NCHAIN), np.float32)
        tbos = np.full((NL, NCHAIN), -10000.0, np.float32)
        for ch in range(NCHAIN):
            s0 = 256 * k + OFF[ch]
            w = WG[ch // 2]
            for j in range(TILES):
                s = s0 - w + j
                if s >= 0:
                    sa[ch, j] = em_t[s]
                if j >= w:
                    sb[ch, j] = M[s]
            if k == 0 and ch == 0:
                gam[:, 0] = 0.0
                tbos[:, 0] = T[BOS, :NL]
        # column order per slot-chunk: [g, t8, chp, b]; slot-major 3D view
        sa = sa.reshape(NGRP, 2, NCH, CHUNK, NL, B)
        sb = sb.reshape(NGRP, 2, NCH, CHUNK, NL, B)
        sa = np.ascontiguousarray(sa.transpose(2, 4, 0, 3, 1, 5)).reshape(NCH, NL, NGRP * CHUNK * 2 * B)
        sb = np.ascontiguousarray(sb.transpose(2, 4, 0, 3, 1, 5)).reshape(NCH, NL, NGRP * CHUNK * 2 * B)

        cb = np.zeros((NL, 2 * NL + 1 + NCHAIN), BF16)
        cb[:, 0:NL] = expT_bf
        cb[:, NL:2 * NL] = np.eye(NL, dtype=BF16)
        cb[:, 2 * NL:2 * NL + 1] = 1.0
        for ch in range(NCHAIN):
            last = (k == NCORES - 1 and ch == NCHAIN - 1)
            cb[:, 2 * NL + 1 + ch] = teos_bf if last else np.ones(NL, BF16)
        cf = np.zeros((NL, 2 * NCHAIN), np.float32)
        cf[:, 0:NCHAIN] = gam
        cf[:, NCHAIN:2 * NCHAIN] = tbos
        in_maps.append({"sa": sa, "sb": sb, "cbf": cb, "cfp": cf})
    return in_maps, K, trans_sc


def _build_program(K):
    import contextlib
    import concourse.bass as bass
    import concourse.tile as tile
    from concourse import bacc, mybir

    dt = mybir.dt
    Alu = mybir.AluOpType
    Act = mybir.ActivationFunctionType
    DR = mybir.MatmulPerfMode.DoubleRow

    GW = NGRP * CHUNK          # slots per chunk across groups
    SLOTW = 2 * B              # columns per slot

    nc = bacc.Bacc("TRN2", target_bir_lowering=False, debug=False, num_devices=NCORES)

    sa_d = nc.dram_tensor("sa", [NCH, NL, GW * SLOTW], dt.float8e4, kind="ExternalInput").ap()
    sb_d = nc.dram_tensor("sb", [NCH, NL, GW * SLOTW], dt.float8e4, kind="ExternalInput").ap()
    cbf_d = nc.dram_tensor("cbf", [NL, 2 * NL + 1 + NCHAIN], dt.bfloat16, kind="ExternalInput").ap()
    cfp_d = nc.dram_tensor("cfp", [NL, 2 * NCHAIN], dt.float32, kind="ExternalInput").ap()

    phis_d = nc.dram_tensor("phis", [1, NCHAIN * 2 * B], dt.float32, kind="ExternalOutput").ap()
    etpart_d = nc.dram_tensor("etpart", [NL, 2], dt.float32, kind="ExternalOutput").ap()

    with tile.TileContext(nc) as tc:
        with contextlib.ExitStack() as ctx:
            const = ctx.enter_context(tc.tile_pool(name="const", bufs=1))
            emring = ctx.enter_context(tc.tile_pool(name="emring", bufs=3))
            exring = ctx.enter_context(tc.tile_pool(name="exring", bufs=3))
            dring = ctx.enter_context(tc.tile_pool(name="dring", bufs=3))
            ps = ctx.enter_context(tc.tile_pool(name="ps", bufs=1, space="PSUM"))
            php = ctx.enter_context(tc.tile_pool(name="php", bufs=2, space="PSUM"))

            negK = const.tile([NL, 1], dt.float32)
            nc.vector.memset(negK[:], -K)
            actwarm = const.tile([NL, 1], dt.float32)
            nc.scalar.activation(actwarm[:], negK[:], Act.Exp, bias=0.0, scale=1.0)

            cbf = const.tile([NL, 2 * NL + 1 + NCHAIN], dt.bfloat16)
            cfp = const.tile([NL, 2 * NCHAIN], dt.float32)
            expT = cbf[:, 0:NL]
            ident = cbf[:, NL:2 * NL]
            fones = cbf[:, 2 * NL:2 * NL + 1]
            fvec = cbf[:, 2 * NL + 1:2 * NL + 1 + NCHAIN]
            gam = cfp[:, 0:NCHAIN]
            tbos = cfp[:, NCHAIN:2 * NCHAIN]

            qg = []
            for g in range(NGRP):
                q = const.tile([NL, SLOTW], dt.bfloat16, name=f"q{g}")
                nc.vector.memset(q[:], 1.0)
                qg.append(q)
            us = [const.tile([NL, B], dt.bfloat16, name=f"u{ch}") for ch in range(NCHAIN)]

            psg = [ps.tile([NL, SLOTW], dt.float32, name=f"psg{g}") for g in range(NGRP)]
            dacc = [ps.tile([NL, NL], dt.float32, name=f"dacc{h}") for h in range(2)]
            phi_sb = const.tile([1, NCHAIN * 2 * B], dt.float32)

            n_dacc = NCH * (CHUNK // 2) * NCHAIN * 2  # DR matmuls total
            i_dacc = 0

            sa_ts, exc_ts = {}, {}

            def fetch_sa(c):
                sa_t = emring.tile([NL, GW * SLOTW], dt.float8e4, name=f"sa{c}", tag="em")
                for g in range(NGRP):
                    sl = slice(g * CHUNK * SLOTW, (g + 1) * CHUNK * SLOTW)
                    nc.sync.dma_start(sa_t[:, sl], sa_d[c, :, sl])
                sa_ts[c] = sa_t
                exc_ts[c] = exring.tile([NL, GW * SLOTW], dt.bfloat16, name=f"ex{c}", tag="ex")

            HSL = CHUNK * SLOTW // 2   # half of a group's chunk block

            def exp_half(c, g, h):
                sl = slice(g * CHUNK * SLOTW + h * HSL, g * CHUNK * SLOTW + (h + 1) * HSL)
                nc.scalar.activation(exc_ts[c][:, sl], sa_ts[c][:, sl],
                                     Act.Exp, bias=negK[:], scale=1.0)

            sb_ts = {}

            def fetch_sb(c):
                sb_t = dring.tile([NL, GW, SLOTW], dt.float8e4, name=f"sb{c}", tag="d")
                nc.sync.dma_start(sb_t[:, :, :], sb_d[c].rearrange("p (w q) -> p w q", w=GW))
                sb_ts[c] = sb_t

            QSL = CHUNK * SLOTW // 4   # quarter of a group's chunk block

            def exp_quarter(c, g, qi):
                sl = slice(g * CHUNK * SLOTW + qi * QSL, g * CHUNK * SLOTW + (qi + 1) * QSL)
                nc.scalar.activation(exc_ts[c][:, sl], sa_ts[c][:, sl],
                                     Act.Exp, bias=negK[:], scale=1.0)

            fetch_sa(0)
            nc.sync.dma_start(cbf[:], cbf_d[:])
            nc.sync.dma_start(cfp[:], cfp_d[:])
            fetch_sb(0)
            # first quarter of every group first, so all three chains start asap;
            # then the remaining three quarters of each group as one op.
            for g in range(NGRP):
                exp_quarter(0, g, 0)
            for ch in range(NCHAIN):
                g, chp = ch // 2, ch % 2
                off = (g * CHUNK + WG[g]) * SLOTW + chp * B
                nc.scalar.activation(us[ch][:], sa_ts[0][:, off:off + B],
                                     Act.Exp, bias=tbos[:, ch:ch + 1], scale=1.0)
            for g in range(NGRP):
                sl = slice(g * CHUNK * SLOTW + QSL, (g + 1) * CHUNK * SLOTW)
                nc.scalar.activation(exc_ts[0][:, sl], sa_ts[0][:, sl],
                                     Act.Exp, bias=negK[:], scale=1.0)

            for c in range(NCH):
                sa_t, exc = sa_ts[c], exc_ts[c]
                sa3 = sa_t[:].rearrange("p (w q) -> p w q", q=SLOTW)
                sb_t = sb_ts[c]
                if c + 1 < NCH:
                    fetch_sa(c + 1)
                    fetch_sb(c + 1)

                for t8 in range(CHUNK):
                    t = c * CHUNK + t8
                    for g in range(NGRP):
                        q, p = qg[g], psg[g]
                        if t == WG[g]:
                            for chp in range(2):
                                ch = 2 * g + chp
                                pht = php.tile([1, B], dt.float32, name=f"php{ch}", tag="ph")
                                nc.tensor.matmul(pht[:], fones[:],
                                                 q[:, chp * B:(chp + 1) * B],
                                                 start=True, stop=True)
                                nc.scalar.copy(phi_sb[:, ch * B:(ch + 1) * B], pht[:])
                        nc.tensor.matmul(p[:], expT[:], q[:], start=True, stop=True)
                        exs = exc[:, (g * CHUNK + t8) * SLOTW:(g * CHUNK + t8 + 1) * SLOTW]
                        nc.vector.tensor_tensor(q[:], p[:], exs, Alu.mult)
                        if t == WG[g]:
                            for chp in range(2):
                                ch = 2 * g + chp
                                qh = q[:, chp * B:(chp + 1) * B]
                                nc.vector.scalar_tensor_tensor(
                                    qh, qh, gam[:, ch:ch + 1],
                                    us[ch][:], Alu.mult, Alu.add)
                    if c + 1 < NCH and t8 in (1, 2, 3, 4, 5, 6):
                        exp_half(c + 1, (t8 - 1) // 2, (t8 - 1) % 2)
                    if t8 % 2 == 1:
                        # DoubleRow diag accumulate over the (t8-1, t8) slot pair
                        sb3 = sb_t
                        for g in range(NGRP):
                            for chp in range(2):
                                for h in range(2):
                                    csl = slice(chp * B + h * NL, chp * B + (h + 1) * NL)
                                    nc.tensor.matmul(
                                        dacc[h][:],
                                        sb3[:, g * CHUNK + t8 - 1: g * CHUNK + t8 + 1, csl],
                                        sa3[:, g * CHUNK + t8 - 1: g * CHUNK + t8 + 1, csl],
                                        start=(i_dacc == 0), stop=(i_dacc == n_dacc - 1),
                                        perf_mode=DR)
                                i_dacc += 1

            # gold-score diag extract first (dacc completes at the last t8)
            escr = const.tile([NL, NL], dt.bfloat16)
            etp = const.tile([NL, 2], dt.float32)
            for h in range(2):
                nc.vector.scalar_tensor_tensor(escr[:], dacc[h][:], 1.0, ident[:],
                                               Alu.mult, Alu.mult, accum_out=etp[:, h:h + 1])
            nc.sync.dma_start(etpart_d[:], etp[:])

            for ch in range(NCHAIN):
                g, chp = ch // 2, ch % 2
                pht = php.tile([1, B], dt.float32, name=f"phe{ch}", tag="ph")
                nc.tensor.matmul(pht[:], fvec[:, ch:ch + 1],
                                 qg[g][:, chp * B:(chp + 1) * B], start=True, stop=True)
                nc.vector.tensor_copy(phi_sb[:, (NCHAIN + ch) * B:(NCHAIN + ch + 1) * B],
                                      pht[:])
            nc.sync.dma_start(phis_d[:], phi_sb[:])

    nc.compile()
    return nc


def run(emissions, tags, transitions, trace=False, trace_cores=None):
    from concourse.bass_utils import run_bass_kernel_spmd
    in_maps, K, trans_sc = _host_prep(emissions, tags, transitions)
    key = f"{K:.9f}"
    if key not in _prog_cache:
        _prog_cache[key] = _build_program(K)
    nc = _prog_cache[key]
    r = run_bass_kernel_spmd(nc, in_maps, list(range(NCORES)), trace=trace,
                             trace_cores=trace_cores)

    # phis per core: [pre0..pre5 | end0..end5] each [B]
    raw = np.stack([r.results[k]["phis"].reshape(2 * NCHAIN, B) for k in range(NCORES)])
    raw = np.log(raw.astype(np.float64))
    pre = raw[:, 0:NCHAIN].reshape(NCORES * NCHAIN, B)
    end = raw[:, NCHAIN:2 * NCHAIN].reshape(NCORES * NCHAIN, B)
    log_z = end[0] + end[1:].sum(0) - pre[1:].sum(0) + 2047.0 * K

    etp = np.stack([r.results[k]["etpart"] for k in range(NCORES)]).sum(0)  # [128, 2]
    em_sc = etp.transpose(1, 0).reshape(2 * NL).astype(np.float64)          # [B]
    scores = em_sc + trans_sc
    nll = -np.mean(scores - log_z) / 100.0
    return np.float32(nll), r


def kernel(emissions, tags, transitions):
    out, _ = run(emissions, tags, transitions, trace=False)
    return out
